# revision 24
# baseline (speedup 1.0000x reference)
"""Causal GQA self-attention (B=2,T=2048,C=4096, 32 q-heads, 8 kv-groups, hs=128)
sharded tensor-parallel across 8 TRN2 NeuronCores: one kv-group (4 q heads) per core.

v2: software-pipelined supersteps over the 8 (batch, 512-token) chunks.
Per superstep s (program order): [proj(s-1)] [qkv(s)+rope+v-transpose] [attn(s)].
- fp16 activations/weights (same PE speed as bf16, 8x mantissa).
- softmax denominator via fp16 row-accumulation R on DVE + one ones-matmul
  per (b,chunk,head) (instead of a full ones-matmul per S tile).
- reciprocal broadcast via tiny fp16 K=1 matmul.
- causal diagonal blocks trimmed (S/exp/mask/O only touch the valid q range).
- qkv m0-matmuls of chunk s+1 issued as PE fillers inside attn(s) to cover
  the ACT-exp pacing deficit; their rope epilogue is deferred to qkv(s+1).
Host sums the 8 partial outputs in fp32.
"""
import math

import numpy as np

import concourse.bass as bass
import concourse.mybir as mybir
import concourse.tile as tile
from concourse import bacc
from concourse.bass_utils import run_bass_kernel_spmd

F16 = mybir.dt.float16
F32 = mybir.dt.float32
AF = mybir.ActivationFunctionType

N_CORES = 8
B, T, C = 2, 2048, 4096
HS = 128
QPK = 4                  # q heads per kv group
GCOLS = (QPK + 2) * HS   # 768 qkv columns per group
TOK = B * T              # 4096
NCH = TOK // 512         # 8 supersteps (b, tcq)
SCALE = float(1.0 / np.sqrt(np.float32(HS)))

FILLERS = True           # qkv-m0 of s+1 as PE fillers during attn(s)
LAZY_CHAIN = True        # defer psd/psb/recip/yT of head h into head h+1's loop

_NC_CACHE = None


def build_nc():
    nc = bacc.Bacc("TRN2", target_bir_lowering=False, debug=False,
                   num_devices=N_CORES)
    xs = nc.dram_tensor("xs", [NCH, 128, 32, 512], F16, kind="ExternalInput").ap()
    wqkv = nc.dram_tensor("wqkv", [C, GCOLS], F16, kind="ExternalInput").ap()
    wproj = nc.dram_tensor("wproj", [QPK * HS, C], F16, kind="ExternalInput").ap()
    cosf = nc.dram_tensor("cosf", [128, T], F16, kind="ExternalInput").ap()
    sinb = nc.dram_tensor("sinb", [128, T], F16, kind="ExternalInput").ap()
    tril = nc.dram_tensor("tril", [128, 128], F16, kind="ExternalInput").ap()
    onesf = nc.dram_tensor("onesf", [128, 128], F16, kind="ExternalInput").ap()
    biasc = nc.dram_tensor("biasc", [128, 1], F32, kind="ExternalInput").ap()
    ident = nc.dram_tensor("ident", [128, 128], F16, kind="ExternalInput").ap()
    out = nc.dram_tensor("out", [TOK, C], F16, kind="ExternalOutput").ap()

    wqkv_r = wqkv.rearrange("(ko p) m -> p ko m", p=128)    # [128, 32, 768]
    wproj_r = wproj.rearrange("(h p) c -> p h c", p=128)    # [128, 4, C]

    with tile.TileContext(nc) as tc:
        with tc.tile_pool(name="const", bufs=1) as cpool, \
             tc.tile_pool(name="xin", bufs=2) as xin, \
             tc.tile_pool(name="qpool", bufs=2) as qpool, \
             tc.tile_pool(name="ypool", bufs=2) as ypool, \
             tc.tile_pool(name="rope", bufs=2) as rpool, \
             tc.tile_pool(name="ptp", bufs=4) as ptp, \
             tc.tile_pool(name="rp", bufs=2) as rp, \
             tc.tile_pool(name="obp", bufs=8) as obp, \
             tc.tile_pool(name="psq", bufs=3, space="PSUM") as psq, \
             tc.tile_pool(name="pss", bufs=2, space="PSUM") as pss, \
             tc.tile_pool(name="pso", bufs=2, space="PSUM") as pso, \
             tc.tile_pool(name="pfl", bufs=1, space="PSUM") as pfl:

            # ---------- persistent SBUF (DMAs ordered for fast start) ----------
            wq_sb = cpool.tile([128, 32, GCOLS], F16)
            tril_sb = cpool.tile([128, 128], F16)
            onesf_sb = cpool.tile([128, 128], F16)
            biasc_sb = cpool.tile([128, 1], F32)
            ident_sb = cpool.tile([128, 128], F16)
            kT = cpool.tile([128, TOK], F16)        # all k, feature-major
            v_tok = cpool.tile([128, 32, 128], F16) # all v, token-major
            wp_sb = cpool.tile([128, 4, C], F16)

            def dma_startup():
                # interleave x0 blocks and wq rows in m0's consumption order
                cos_c = xin.tile([128, 512], F16, tag="cos", name="cos0")
                sin_c = xin.tile([128, 512], F16, tag="sin", name="sin0")
                nc.sync.dma_start(cos_c[:], cosf[:, 0:512])
                nc.sync.dma_start(sin_c[:], sinb[:, 0:512])
                cs_tiles[0] = (cos_c, sin_c)
                xa = xin.tile([128, 16, 512], F16, tag="xa", name="xa0")
                xb = xin.tile([128, 16, 512], F16, tag="xb", name="xb0")
                x_tiles[0] = (xa, xb)
                nc.sync.dma_start(xa[:, 0:8, :], xs[0, :, 0:8, :])
                for kk in range(8):
                    nc.sync.dma_start(wq_sb[:, kk, :], wqkv_r[:, kk, :])
                nc.sync.dma_start(xa[:, 8:16, :], xs[0, :, 8:16, :])
                for kk in range(8, 16):
                    nc.sync.dma_start(wq_sb[:, kk, :], wqkv_r[:, kk, :])
                nc.sync.dma_start(xb[:], xs[0, :, 16:32, :])
                for kk in range(16, 32):
                    nc.sync.dma_start(wq_sb[:, kk, :], wqkv_r[:, kk, :])
                nc.sync.dma_start(tril_sb[:], tril[:])
                nc.sync.dma_start(onesf_sb[:], onesf[:])
                nc.sync.dma_start(biasc_sb[:], biasc[:])
                nc.sync.dma_start(ident_sb[:], ident[:])

            def dma_wproj():
                for hh in range(4):
                    nc.gpsimd.dma_start(wp_sb[:, hh, :], wproj_r[:, hh, :])

            # per-superstep state handed across phases
            cs_tiles = [None] * NCH    # (cos_c, sin_c) per chunk
            x_tiles = [None] * NCH     # (xa, xb) SBUF tiles per chunk
            qT_tiles = [None] * NCH    # [128, 4, 512] fp16 q (roped)
            yT_tiles = [None] * NCH    # [128, 4, 512] fp16 attention out
            fill_state = {}            # chunk -> (pfill_tile, n_mms_emitted)

            def dma_x(s):
                t0 = s * 512
                tb = t0 % T
                cos_c = xin.tile([128, 512], F16, tag="cos", name=f"cos{s}")
                sin_c = xin.tile([128, 512], F16, tag="sin", name=f"sin{s}")
                nc.sync.dma_start(cos_c[:], cosf[:, tb:tb + 512])
                nc.sync.dma_start(sin_c[:], sinb[:, tb:tb + 512])
                cs_tiles[s] = (cos_c, sin_c)
                xa = xin.tile([128, 16, 512], F16, tag="xa", name=f"xa{s}")
                xb = xin.tile([128, 16, 512], F16, tag="xb", name=f"xb{s}")
                nc.sync.dma_start(xa[:], xs[s, :, 0:16, :])
                nc.sync.dma_start(xb[:], xs[s, :, 16:32, :])
                x_tiles[s] = (xa, xb)

            def emit_filler_mm(s):
                """One qkv m=0 matmul for chunk s into the pfill psum tile."""
                st = fill_state.get(s)
                if st is None:
                    pf = pfl.tile([128, 512], F32, tag="f", name=f"pf{s}")
                    st = fill_state[s] = [pf, 0]
                pf, kk = st
                if kk >= 32:
                    return False
                xa, xb = x_tiles[s]
                xt_t = xa if kk < 16 else xb
                nc.tensor.matmul(pf[:], wq_sb[:, kk, 0:128], xt_t[:, kk % 16, :],
                                 start=(kk == 0), stop=(kk == 31))
                st[1] = kk + 1
                return True

            def rope_epilogue(s, m, ps):
                """psum [128,512] f32 -> roped fp16 into qT/kT (or copy for v)."""
                t0 = s * 512
                tb = t0 % T
                if m == 5:
                    vt = rpool.tile([128, 512], F16, tag="vt", name=f"vt{s}")
                    nc.vector.tensor_copy(vt[:], ps[:])
                    return vt
                cos_c, sin_c = cs_tiles[s]
                t1 = rpool.tile([128, 512], F32, tag="t1", name=f"t1_{s}_{m}")
                nc.vector.tensor_mul(t1[:], ps[:], cos_c[:])
                u = rpool.tile([128, 512], F32, tag="u", name=f"u_{s}_{m}")
                nc.vector.tensor_mul(u[0:64, :], ps[64:128, :], sin_c[0:64, :])
                nc.vector.tensor_mul(u[64:128, :], ps[0:64, :], sin_c[64:128, :])
                if m < 4:
                    dst = qT_tiles[s][:, m, :]
                else:
                    dst = kT[:, t0:t0 + 512]
                nc.vector.tensor_add(dst, t1[:], u[:])
                return None

            def emit_qkv(s, pending=None):
                pending = list(pending or [])
                n_mm = [0]

                def tick():
                    n_mm[0] += 1
                    if pending and n_mm[0] in (8, 16):
                        pending.pop(0)()
                if s + 1 < NCH:
                    dma_x(s + 1)
                qT_tiles[s] = qpool.tile([128, 4, 512], F16, tag="q", name=f"qT{s}")
                xa, xb = x_tiles[s]
                vt = None
                st = fill_state.pop(s, None)
                m_start = 0
                if st is not None and st[1] == 32:
                    rope_epilogue(s, 0, st[0])
                    m_start = 1
                elif st is not None:
                    # partially filled m0: finish it here
                    pf = st[0]
                    while st[1] < 32:
                        kk = st[1]
                        xt_t = xa if kk < 16 else xb
                        nc.tensor.matmul(pf[:], wq_sb[:, kk, 0:128],
                                         xt_t[:, kk % 16, :],
                                         start=(kk == 0), stop=(kk == 31))
                        st[1] = kk + 1
                        tick()
                    rope_epilogue(s, 0, pf)
                    m_start = 1
                for m in range(m_start, 6):
                    ps = psq.tile([128, 512], F32, tag="g", name=f"qkv{s}_{m}")
                    for kk in range(32):
                        xt_t = xa if kk < 16 else xb
                        nc.tensor.matmul(ps[:], wq_sb[:, kk, m * 128:(m + 1) * 128],
                                         xt_t[:, kk % 16, :],
                                         start=(kk == 0), stop=(kk == 31))
                        tick()
                    r = rope_epilogue(s, m, ps)
                    if r is not None:
                        vt = r
                while pending:
                    pending.pop(0)()
                # v -> token-major via PE transpose
                for j in range(4):
                    tp = pss.tile([128, 128], F16, tag="s", name=f"tp{s}_{j}")
                    nc.tensor.transpose(tp[:], vt[:, j * 128:(j + 1) * 128],
                                        ident_sb[:])
                    nc.vector.tensor_copy(v_tok[:, s * 4 + j, :], tp[:])
                x_tiles[s] = None

            def emit_proj(s, pending=None):
                """proj for chunk s tokens; copies split scalar/vector.
                pending: deferred h3 normalize chain from attn(s), popped
                between early tiles so its latency hides under proj MMs."""
                t0 = s * 512
                yT = yT_tiles[s]
                pending = list(pending or [])
                tiles = [(ti, cc) for ti in range(4) for cc in range(8)]
                psps = {}

                def mm_tile(key, h_range):
                    ti, cc = key
                    psp = psps.get(key)
                    if psp is None:
                        pool_, tag_ = ((pfl, "f") if (ti * 8 + cc) % 4 == 3
                                       else (psq, "g"))
                        psp = psps[key] = pool_.tile(
                            [128, 512], F32, tag=tag_, name=f"pj{s}_{ti}_{cc}")
                    for h in h_range:
                        nc.tensor.matmul(
                            psp[:], yT[:, h, ti * 128:(ti + 1) * 128],
                            wp_sb[:, h, cc * 512:(cc + 1) * 512],
                            start=(h == 0), stop=(h == 3))

                def drain_tile(key):
                    ti, cc = key
                    psp = psps.pop(key)
                    ob = obp.tile([128, 512], F16, tag="ob",
                                  name=f"ob{s}_{ti}_{cc}")
                    if (ti * 8 + cc) % 2 == 0:
                        nc.scalar.copy(ob[:], psp[:])
                    else:
                        nc.vector.tensor_copy(ob[:], psp[:])
                    nc.sync.dma_start(
                        out[t0 + ti * 128:t0 + ti * 128 + 128,
                            cc * 512:(cc + 1) * 512], ob[:])

                if pending:
                    mm_tile(tiles[0], range(3))
                    pending.pop(0)()          # d3: ones-MM + recip
                    mm_tile(tiles[1], range(3))
                    pending.pop(0)()          # b3: yT(h3) normalize (DVE)
                    mm_tile(tiles[0], range(3, 4))
                    drain_tile(tiles[0])
                    mm_tile(tiles[1], range(3, 4))
                    drain_tile(tiles[1])
                    tiles = tiles[2:]
                for key in tiles:
                    mm_tile(key, range(QPK))
                    drain_tile(key)
                yT_tiles[s] = None

            def emit_attn(s):
                b, tcq = s // 4, s % 4
                t0g = b * T + tcq * 512
                n_s = (tcq + 1) * 4
                qT = qT_tiles[s]
                yT_tiles[s] = ypool.tile([128, 4, 512], F16, tag="y",
                                         name=f"yT{s}")
                yT = yT_tiles[s]
                pending = []   # deferred chain closures from previous head

                def pop_pending(k=1):
                    for _ in range(k):
                        if pending:
                            pending.pop(0)()

                def filler():
                    if FILLERS and s + 1 < NCH:
                        emit_filler_mm(s + 1)

                for h in range(QPK):
                    ps_o = pso.tile([128, 512], F32, tag="o", name=f"o{s}_{h}")
                    R = rp.tile([128, 512], F16, tag="r", name=f"R{s}_{h}")
                    for si in range(n_s):
                        j = si - tcq * 4          # >=0 on diagonal blocks
                        off = 128 * j if j >= 0 else 0
                        s0g = b * T + si * 128
                        ps_s = pss.tile([128, 512], F32, tag="s",
                                        name=f"s{s}_{h}_{si}")
                        nc.tensor.matmul(ps_s[:, off:512], kT[:, s0g:s0g + 128],
                                         qT[:, h, off:512],
                                         start=True, stop=True)
                        if si % 2 == 0:
                            pop_pending()
                        filler()
                        pt = ptp.tile([128, 512], F16, tag="pt",
                                      name=f"pt{s}_{h}_{si}")
                        nc.scalar.activation(pt[:, off:512], ps_s[:, off:512],
                                             AF.Exp, scale=SCALE, bias=biasc_sb[:])
                        if j >= 0:
                            nc.vector.tensor_mul(pt[:, off:off + 128],
                                                 pt[:, off:off + 128], tril_sb[:])
                        if si == 0:
                            nc.vector.tensor_copy(R[:], pt[:])
                        else:
                            nc.vector.tensor_add(R[:, off:512], R[:, off:512],
                                                 pt[:, off:512])
                        nc.tensor.matmul(ps_o[:, off:512],
                                         v_tok[:, b * 16 + si, :],
                                         pt[:, off:512],
                                         start=(si == 0), stop=(si == n_s - 1),
                                         skip_group_check=True)

                    def make_chain(h=h, ps_o=ps_o, R=R):
                        steps = []

                        def d_step():
                            # ones [128,128] stationary: every output partition
                            # gets the column sum -> denominator pre-broadcast
                            psd = psq.tile([128, 512], F32, tag="g",
                                           name=f"d{s}_{h}")
                            nc.tensor.matmul(psd[:], onesf_sb[:], R[:],
                                             start=True, stop=True)
                            rb = rpool.tile([128, 512], F32, tag="t1",
                                            name=f"rb{s}_{h}")
                            nc.vector.reciprocal_approx_fast(rb[:], psd[:])
                            d_step.rb = rb

                        def b_step():
                            nc.vector.tensor_mul(yT[:, h, :], ps_o[:],
                                                 d_step.rb[:])

                        steps.append(d_step)
                        steps.append(b_step)
                        return steps

                    if LAZY_CHAIN:
                        pending.extend(make_chain())
                    else:
                        for st_fn in make_chain():
                            st_fn()
                if FILLERS and s + 1 < NCH:
                    st = fill_state.get(s + 1)
                    while st is None or st[1] < 32:
                        if not emit_filler_mm(s + 1):
                            break
                        st = fill_state.get(s + 1)
                return pending

            # ---------------- main pipeline ----------------
            dma_startup()
            chains = {}
            for s in range(NCH):
                emit_qkv(s, chains.pop(s - 1, None) if s >= 1 else None)
                if s == 0:
                    dma_wproj()
                if s >= 1:
                    emit_proj(s - 1)
                chains[s] = emit_attn(s)
            emit_proj(NCH - 1, chains.pop(NCH - 1, None))

    nc.compile()
    return nc


def _prep_inputs(x, cos, sin, Wqkv, Wproj):
    f16 = np.float16
    xTn = x.reshape(TOK, C).T.astype(f16)      # [C, TOK]
    xsn = np.ascontiguousarray(
        xTn.reshape(32, 128, NCH, 512).transpose(2, 1, 0, 3))
    cosf = np.ascontiguousarray(cos.T.astype(f16))        # [128, T]
    sinT = sin.T.astype(np.float32)
    sinb = np.concatenate([-sinT[0:64], sinT[64:128]], axis=0)
    sinb = np.ascontiguousarray(sinb).astype(f16)
    p = np.arange(128)[:, None]
    f = np.arange(128)[None, :]
    tril = (p <= f).astype(f16)
    onesf = np.ones([128, 128], dtype=f16)
    biasc = np.full([128, 1], -6.0, dtype=np.float32)
    ident = np.eye(128, dtype=f16)
    in_maps = []
    for g in range(N_CORES):
        in_maps.append({
            "xs": xsn,
            "wqkv": np.ascontiguousarray(Wqkv[:, g * GCOLS:(g + 1) * GCOLS]).astype(f16),
            "wproj": np.ascontiguousarray(Wproj[g * 512:(g + 1) * 512, :]).astype(f16),
            "cosf": cosf, "sinb": sinb, "tril": tril,
            "onesf": onesf, "biasc": biasc, "ident": ident,
        })
    return in_maps


def kernel(x, cos, sin, Wqkv, Wproj, _trace=False):
    global _NC_CACHE
    x = np.asarray(x, dtype=np.float32)
    cos = np.asarray(cos, dtype=np.float32)
    sin = np.asarray(sin, dtype=np.float32)
    Wqkv = np.asarray(Wqkv, dtype=np.float32)
    Wproj = np.asarray(Wproj, dtype=np.float32)
    if _NC_CACHE is None:
        _NC_CACHE = build_nc()
    nc = _NC_CACHE
    in_maps = _prep_inputs(x, cos, sin, Wqkv, Wproj)
    res = run_bass_kernel_spmd(nc, in_maps, core_ids=list(range(N_CORES)),
                               trace=_trace)
    acc = np.zeros([TOK, C], dtype=np.float32)
    for r in res.results:
        acc += r["out"].astype(np.float32)
    if _trace:
        kernel._last_exec_ns = res.exec_time_ns
        kernel._last_trace = res.instructions_and_trace
    return acc.reshape(B, T, C)
